# revision 2
# baseline (speedup 1.0000x reference)
"""GroupedQueryAttention Trainium2 kernel (8-core SPMD), v2.

Reference op: RMSNorm -> in-proj (q/k/v) -> RoPE -> causal GQA attention
-> out-proj -> residual.  b=2, s=2048, d_model=2048, 32 q-heads / 8 KV
groups, head dim 64, fp32.

Sharding: core c handles batch b = c//4 and KV groups (2j, 2j+1), j = c%4
(data parallel over batch x tensor parallel over KV groups, Megatron
style).  Each core computes the full in-projection restricted to its 8
heads' channels, attention for its 8 heads, and a partial out-projection
(row-parallel).  The host sums the 4 partials per batch (f16 partials,
f32 sum) and adds the residual.

v2 structure (vs the v1 baseline):
  * Three-stage software pipeline: chunk c runs in-proj(c), attention
    (c-1) and out-proj(c-2) concurrently, with the PE instruction stream
    explicitly woven so matmuls from in/out-proj fill the gaps where
    attention waits on the ACT engine's exp.
  * PSUM plan (8 banks): ip 1 + qk 4 (double-buffered [128,2,512]) +
    av 2 + op 1.  The RMS sum-of-squares psum rides the qk tag early;
    the out-proj tail ping-pongs op/qk tags once attention is done.
  * Engine balance: exp on ACT; causal tri-masking on GPSIMD (otherwise
    idle); everything f16 on DVE where possible (2x rate); softmax
    reciprocals via reciprocal_approx_fast straight off the PSUM
    denominator rows; RMS norm uses one Sqrt phase up front so the ACT
    table set switches exactly once (sqrt -> exp).
  * RoPE as 3 DVE ops per tile: the rotate-half operand is produced by
    4 small SBUF->SBUF partition-shift DMAs instead of 4 extra DVE ops,
    with the sign pattern folded into the sin table.
  * w_out resident in SBUF; output written f16 (halves the store
    traffic; host sums partials in f32).
"""

import numpy as np
from collections import deque
from contextlib import ExitStack

import concourse.bass as bass
from concourse import bacc as _bacc
import concourse.mybir as mybir
import concourse.tile as tile
from concourse.bass import ts

import os
f32 = mybir.dt.float32
f16 = mybir.dt.float16
MDT = {"f16": f16, "bf16": mybir.dt.bfloat16}[os.environ.get("GQA_MM_DT", "f16")]
try:
    import ml_dtypes
    _BF16_NP = ml_dtypes.bfloat16
except ImportError:
    _BF16_NP = None
MDT_NP = {f16: np.float16, mybir.dt.bfloat16: _BF16_NP}[MDT]
AF = mybir.ActivationFunctionType
ALU = mybir.AluOpType

TRI_ENGINE = os.environ.get("GQA_TRI_ENGINE", "gpsimd")

D = 2048          # model dim
CH = 768          # per-core in-proj channels (8 q heads + 2 k + 2 v)
TOKC = 512        # token chunk
NKT = D // 128    # 16 k-tiles over model dim
RMS_EPS = 1e-6
ROPE_THETA = 10000.0
NCORES = 8


def build_program(S=2048):
    NCH = S // TOKC          # token chunks
    NSK = S // 128           # sk tiles
    nc = _bacc.Bacc(None)

    xT_d = nc.dram_tensor("xT", [D, S], MDT, kind="ExternalInput")
    w_inT_d = nc.dram_tensor("w_inT", [D, CH], MDT, kind="ExternalInput")
    w_outT_d = nc.dram_tensor("w_outT", [512, D], MDT, kind="ExternalInput")
    cos_d = nc.dram_tensor("cos_t", [128, S], MDT, kind="ExternalInput")
    sin_d = nc.dram_tensor("sin_t", [128, S], MDT, kind="ExternalInput")
    tri_d = nc.dram_tensor("tri", [128, 128], MDT, kind="ExternalInput")
    oner_d = nc.dram_tensor("oner", [1], MDT, kind="ExternalInput")
    eps_d = nc.dram_tensor("epsc", [1], f32, kind="ExternalInput")
    yT_d = nc.dram_tensor("yT", [D, S], MDT, kind="ExternalOutput")

    mask_eng = None  # set inside

    with tile.TileContext(nc) as tc, ExitStack() as ctx:
        sb = ctx.enter_context(tc.tile_pool(name="sb", bufs=1))
        sbs = ctx.enter_context(tc.tile_pool(name="sbs", bufs=2))
        dramp = ctx.enter_context(tc.tile_pool(name="dram", bufs=1, space="DRAM"))

        mask_eng = nc.gpsimd if TRI_ENGINE == "gpsimd" else nc.vector

        # ------------- persistent SBUF -------------
        w_in_sb = sb.tile([128, NKT, CH], MDT, name="w_in_sb")
        w_out_sb = sb.tile([128, 4, D], MDT, name="w_out_sb")
        x_sb = sb.tile([128, NCH * NKT, TOKC], MDT, name="x_sb")
        qkv = sb.tile([128, 6, S], MDT, name="qkv")     # ch tiles 0-3 q pairs, 4 k, 5 v
        oT = sb.tile([128, 4, S], MDT, name="oT")
        vA = sb.tile([128, NSK, 65], MDT, name="vA")    # V + ones col, group 0
        vB = sb.tile([128, NSK, 65], MDT, name="vB")    # group 1
        tri_sb = sb.tile([128, 128], MDT, name="tri_sb")
        ones_sb = sb.tile([128, 1], MDT, name="ones_sb")
        eps_sb = sb.tile([1, 1], f32, name="eps_sb")
        invT = sb.tile([128, NCH, TOKC // 128], f32, name="invT")
        inv128h = sb.tile([128, NCH, TOKC], MDT, name="inv128h")

        nrm_dr = dramp.tile([NCH, TOKC], f32, name="nrm_dr")
        nrm2h_dr = dramp.tile([NCH, TOKC], MDT, name="nrm2h_dr")
        db2_dr = dramp.tile([NCH, 4, 2, TOKC], f32, name="db2_dr")

        # ------------- static loads -------------
        w_inT_v = w_inT_d.rearrange("(o p) c -> p o c", p=128)
        for kt in range(NKT):
            nc.sync.dma_start(w_in_sb[:, kt, :], w_inT_v[:, kt, :])
        w_outT_v = w_outT_d.rearrange("(o p) c -> p o c", p=128)
        for kt in range(4):
            nc.sync.dma_start(w_out_sb[:, kt, :], w_outT_v[:, kt, :])
        nc.sync.dma_start(tri_sb[:], tri_d[:])
        nc.sync.dma_start(ones_sb[:], oner_d[None, :].to_broadcast((128, 1)))
        nc.sync.dma_start(vA[:, :, 64:65], oner_d[None, None, :].to_broadcast((128, NSK, 1)))
        nc.sync.dma_start(vB[:, :, 64:65], oner_d[None, None, :].to_broadcast((128, NSK, 1)))
        nc.sync.dma_start(eps_sb[:], eps_d[None, :])
        for c in range(NCH):
            for kt in range(NKT):
                nc.sync.dma_start(x_sb[:, c * NKT + kt, :],
                                  xT_d[ts(kt, 128), c * TOKC:(c + 1) * TOKC])

        with tc.tile_pool(name="ps", bufs=1, space="PSUM") as ps:

            # ---------------- RMS-norm chain for one chunk ----------------
            def ss_items(c):
                """PE filler items (2 matmuls each) for chunk c's sum of
                squares, plus the trailing scalar chain."""
                ss = ps.tile([1, TOKC], f32, tag="qk", bufs=2, name=f"ss_{c}")

                def mk(j):
                    def emit():
                        for kt in (2 * j, 2 * j + 1):
                            xsq = sbs.tile([128, TOKC], MDT, tag="xsq", bufs=2,
                                           name=f"xsq_{c}_{kt}")
                            nc.vector.tensor_tensor(
                                xsq[:], x_sb[:, c * NKT + kt, :],
                                x_sb[:, c * NKT + kt, :], ALU.mult)
                            nc.tensor.matmul(ss[:], ones_sb[:], xsq[:],
                                             start=(kt == 0), stop=(kt == NKT - 1))
                    return emit

                def post():
                    # inv_rms = 1/sqrt(ss/D + eps); reciprocal runs in a
                    # [128, 4] token-partition layout via a DRAM bounce.
                    sqm = sbs.tile([1, TOKC], f32, tag="sqm", bufs=2, name=f"sqm_{c}")
                    nc.scalar.activation(sqm[:], ss[:], AF.Sqrt,
                                         bias=eps_sb[:], scale=1.0 / D)
                    nc.sync.dma_start(nrm_dr[c][None, :], sqm[:])
                    srT = sbs.tile([128, TOKC // 128], f32, tag="srT", bufs=2,
                                   name=f"srT_{c}")
                    nc.sync.dma_start(srT[:], nrm_dr[c].rearrange("(a p) -> p a", p=128))
                    nc.vector.reciprocal_approx_fast(invT[:, c, :], srT[:])
                    invTh = sbs.tile([128, TOKC // 128], MDT, tag="invTh", bufs=2,
                                     name=f"invTh_{c}")
                    nc.vector.tensor_copy(invTh[:], invT[:, c, :])
                    nc.sync.dma_start(nrm2h_dr[c].rearrange("(a p) -> p a", p=128),
                                      invTh[:])
                    nc.sync.dma_start(inv128h[:, c, :],
                                      nrm2h_dr[c][None, :].to_broadcast((128, TOKC)))

                return [mk(j) for j in range(NKT // 2)], post

            # ---------------- in-projection m-tile ----------------
            def ip_items(c, m):
                cs = slice(c * TOKC, (c + 1) * TOKC)
                ip = ps.tile([128, TOKC], f32, tag="ip", bufs=1, name=f"ip{m}_{c}")

                def mk(j):
                    def emit():
                        for kt in (2 * j, 2 * j + 1):
                            nc.tensor.matmul(ip[:], w_in_sb[:, kt, ts(m, 128)],
                                             x_sb[:, c * NKT + kt, :],
                                             start=(kt == 0), stop=(kt == NKT - 1))
                    return emit

                def post():
                    nc.vector.tensor_copy(qkv[:, m, cs], ip[:])
                    if m < 5:
                        # rope in place: rotate-half operand via partition-shift
                        # DMAs, sign pattern folded into the sin table.
                        qs = sbs.tile([128, TOKC], MDT, tag="qs", bufs=2,
                                      name=f"qs_{c}_{m}")
                        for dst, src in ((0, 32), (32, 0), (64, 96), (96, 64)):
                            nc.sync.dma_start(qs[dst:dst + 32, :],
                                              qkv[src:src + 32, m, cs])
                        rtmp = sbs.tile([128, TOKC], MDT, tag="rtmp", bufs=2,
                                        name=f"rtmp_{c}_{m}")
                        nc.vector.tensor_tensor(rtmp[:], qs[:], sini_t[c][:], ALU.mult)
                        nc.vector.tensor_tensor(qkv[:, m, cs], qkv[:, m, cs],
                                                cosi_t[c][:], ALU.mult)
                        nc.vector.tensor_tensor(qkv[:, m, cs], qkv[:, m, cs],
                                                rtmp[:], ALU.add)
                    else:
                        # V: transpose to [token, dv] (+ 1/rms per-token scale)
                        for tl in range(TOKC // 128):
                            t = c * (TOKC // 128) + tl
                            vtt = sbs.tile([128, 128], MDT, tag="vtt", bufs=2,
                                           name=f"vtt_{t}")
                            nc.sync.dma_start(vtt[:], qkv[:, 5, ts(t, 128)],
                                              transpose=True)
                            nc.scalar.activation(vA[:, t, 0:64], vtt[:, 0:64],
                                                 AF.Copy, scale=invT[:, c, tl:tl + 1])
                            nc.scalar.activation(vB[:, t, 0:64], vtt[:, 64:128],
                                                 AF.Copy, scale=invT[:, c, tl:tl + 1])

                return [mk(j) for j in range(NKT // 2)], post

            # ---------------- out-projection m-tile ----------------
            def op_item(o, m, tag="op"):
                os_ = slice(o * TOKC, (o + 1) * TOKC)

                def emit():
                    op = ps.tile([128, TOKC], f32, tag=tag,
                                 bufs=(1 if tag == "op" else 2), name=f"op_{o}_{m}")
                    for kt in range(4):
                        nc.tensor.matmul(op[:], w_out_sb[:, kt, ts(m, 128)],
                                         oT[:, kt, os_],
                                         start=(kt == 0), stop=(kt == 3))
                    yt = sbs.tile([128, TOKC], MDT, tag="yt", bufs=3,
                                  name=f"yt_{o}_{m}")
                    nc.vector.tensor_copy(yt[:], op[:])
                    nc.sync.dma_start(yT_d[ts(m, 128), os_], yt[:])
                return emit

            # ---------------- attention pair (spine) ----------------
            def attn_pair(a, p, items):
                """Emit the attention chain for pair p of chunk a, draining
                `items` (a deque of PE-work closures) between the QK and AV
                matmul groups so the PE never idles on the ACT exp."""
                cs = slice(a * TOKC, (a + 1) * TOKC)
                n_t = 4 * (a + 1)
                avA = ps.tile([65, TOKC], f32, tag="av", bufs=2, name=f"avA_{a}_{p}")
                avB = ps.tile([65, TOKC], f32, tag="av", bufs=2, name=f"avB_{a}_{p}")
                for t in range(n_t):
                    j0 = max(0, t - 4 * a) * 128
                    qk = ps.tile([128, 2, TOKC], f32, tag="qk", bufs=2,
                                 name=f"qk_{a}_{p}_{t}")
                    # the pair's two heads: row-tiled concurrent K=64 matmuls
                    nc.tensor.matmul(
                        qk[:, 0, j0:],
                        qkv[0:64, 4, ts(t, 128)],
                        qkv[0:64, p, a * TOKC + j0:(a + 1) * TOKC],
                        start=True, stop=True,
                    )
                    nc.tensor.matmul(
                        qk[:, 1, j0:],
                        qkv[64:128, 4, ts(t, 128)],
                        qkv[64:128, p, a * TOKC + j0:(a + 1) * TOKC],
                        start=True, stop=True,
                    )
                    e = sbs.tile([128, 2, TOKC], MDT, tag="e", bufs=3,
                                 name=f"e_{a}_{p}_{t}")
                    nc.scalar.activation(e[:, :, j0:], qk[:, :, j0:], AF.Exp)
                    if t >= 4 * a:  # diagonal tile: causal mask
                        for h in (0, 1):
                            mask_eng.tensor_tensor(
                                e[:, h, j0:j0 + 128],
                                e[:, h, j0:j0 + 128],
                                tri_sb[:],
                                ALU.mult,
                            )
                    # drain filler so the PE has work while ACT runs exp
                    k = -(-len(items) // (n_t - t)) if items else 0
                    for _ in range(min(k, 2 + (len(items) > 2 * (n_t - t)))):
                        if items:
                            items.popleft()()
                    nc.tensor.matmul(avA[:, j0:], vA[:, t, :], e[:, 0, j0:],
                                     start=(t == 0), stop=(t == n_t - 1))
                    nc.tensor.matmul(avB[:, j0:], vB[:, t, :], e[:, 1, j0:],
                                     start=(t == 0), stop=(t == n_t - 1))
                # softmax denominators live in row 64 of each AV psum.
                d2i = sbs.tile([1, 2, TOKC], f32, tag="d2i", bufs=2,
                               name=f"d2i_{a}_{p}")
                nc.vector.reciprocal_approx_fast(d2i[:, 0, :], avA[64:65, :])
                nc.vector.reciprocal_approx_fast(d2i[:, 1, :], avB[64:65, :])
                nc.sync.dma_start(db2_dr[a, p], d2i[0])
                dbAB = sbs.tile([128, TOKC], f32, tag="dbAB", bufs=2,
                                name=f"dbAB_{a}_{p}")
                nc.sync.dma_start(
                    dbAB[0:64, :], db2_dr[a, p, 0][None, :].to_broadcast((64, TOKC)))
                nc.sync.dma_start(
                    dbAB[64:128, :], db2_dr[a, p, 1][None, :].to_broadcast((64, TOKC)))
                # evacuate the AV psum immediately (frees the banks for the
                # next pair), normalize in place in SBUF.
                nc.vector.tensor_copy(oT[0:64, p, cs], avA[0:64, :])
                nc.vector.tensor_copy(oT[64:128, p, cs], avB[0:64, :])
                nc.vector.tensor_tensor(oT[:, p, cs], oT[:, p, cs], dbAB[:],
                                        ALU.mult)

            # ---------------- chunk-level schedule ----------------
            # cosi/sini per chunk (rope tables scaled by 1/rms)
            cosi_t = {}
            sini_t = {}

            def emit_chunk_tables(c):
                cos_c = sbs.tile([128, TOKC], MDT, tag="cos_c", bufs=2,
                                 name=f"cos_c_{c}")
                nc.sync.dma_start(cos_c[:], cos_d[:, c * TOKC:(c + 1) * TOKC])
                sin_c = sbs.tile([128, TOKC], MDT, tag="sin_c", bufs=2,
                                 name=f"sin_c_{c}")
                nc.sync.dma_start(sin_c[:], sin_d[:, c * TOKC:(c + 1) * TOKC])
                cosi = sbs.tile([128, TOKC], MDT, tag="cosi", bufs=2,
                                name=f"cosi_{c}")
                nc.vector.tensor_tensor(cosi[:], cos_c[:], inv128h[:, c, :], ALU.mult)
                sini = sbs.tile([128, TOKC], MDT, tag="sini", bufs=2,
                                name=f"sini_{c}")
                nc.vector.tensor_tensor(sini[:], sin_c[:], inv128h[:, c, :], ALU.mult)
                cosi_t[c] = cosi
                sini_t[c] = sini

            # chunk-0 norm chain runs first (gates everything)
            items0, post0 = ss_items(0)
            for it in items0:
                it()
            post0()
            emit_chunk_tables(0)

            # op m-tiles per step: 16 spread over 6 steps
            OP_STEPS = [range(0, 3), range(3, 6), range(6, 9), range(9, 12),
                        range(12, 14), range(14, 16)]

            for c in range(NCH + 2):
                a = c - 1   # attention chunk
                o = c - 2   # out-proj chunk
                if 0 < c < NCH:
                    emit_chunk_tables(c)
                for s in range(6):
                    items = deque()
                    posts = []
                    if c < NCH:
                        its, post = ip_items(c, s)
                        items.extend(its)
                        posts.append(post)
                    if c == 0 and 1 <= s <= 3:
                        its, post = ss_items(s)
                        items.extend(its)
                        posts.append(post)
                    if 0 <= o < NCH and c < NCH + 1:
                        for m in OP_STEPS[s]:
                            items.append(op_item(o, m))
                    if c == NCH + 1:
                        # tail: out-proj of the last chunk ping-pongs between
                        # the op and (now free) qk psum tags
                        for i, m in enumerate(OP_STEPS[s]):
                            items.append(op_item(o, m, tag=("op" if (m % 2 == 0)
                                                            else "qk")))
                    if 0 <= a < NCH and s < 4:
                        attn_pair(a, s, items)
                    while items:
                        items.popleft()()
                    for post in posts:
                        post()

    nc.finalize()
    return nc


# ------------------------------- host side ----------------------------------

def _rope_tables(S):
    inv_freq = ROPE_THETA ** (-np.arange(0, 64, 2, dtype=np.float64) / 64.0)  # [32]
    ang = np.arange(S, dtype=np.float64)[:, None] * inv_freq[None, :]          # [S, 32]
    cosb = np.cos(ang).T.astype(np.float32)   # [32, S]
    sinb = np.sin(ang).T.astype(np.float32)
    cos128 = np.tile(cosb, (4, 1))                                             # [128, S]
    # sign pattern for the pre-swapped rotate-half operand:
    # out[0:32] needs -x2*s, out[32:64] needs +x1*s (per 64-row head block)
    sin128 = np.concatenate([-sinb, sinb, -sinb, sinb], axis=0)                # [128, S]
    return (np.ascontiguousarray(cos128).astype(MDT_NP),
            np.ascontiguousarray(sin128).astype(MDT_NP))


def host_prepare(x, w_in, w_out, rms_w):
    """Build the 8 per-core input maps."""
    S = x.shape[1]
    x = np.asarray(x, dtype=np.float32)
    w_eff = np.asarray(w_in, dtype=np.float32) * np.asarray(rms_w, np.float32)[None, :]
    w_out = np.asarray(w_out, dtype=np.float32)
    cos128, sin128 = _rope_tables(S)
    tri = np.ascontiguousarray(np.triu(np.ones((128, 128), dtype=np.float32)))
    qscale = np.float32(64 ** -0.5)

    in_maps = []
    for core in range(NCORES):
        b, j = divmod(core, 4)
        g0, g1 = 2 * j, 2 * j + 1
        rows = []
        for p in range(4):
            for g in (g0, g1):
                rows.extend(range((g * 4 + p) * 64, (g * 4 + p) * 64 + 64))
        for g in (g0, g1):
            rows.extend(range(2048 + g * 64, 2048 + g * 64 + 64))
        for g in (g0, g1):
            rows.extend(range(2560 + g * 64, 2560 + g * 64 + 64))
        w_slice = w_eff[rows, :].copy()
        w_slice[:512, :] *= qscale
        cols = []
        for p in range(4):
            for g in (g0, g1):
                cols.extend(range((g * 4 + p) * 64, (g * 4 + p) * 64 + 64))
        in_maps.append({
            "xT": np.ascontiguousarray(x[b].T).astype(MDT_NP),
            "w_inT": np.ascontiguousarray(w_slice.T).astype(MDT_NP),
            "w_outT": np.ascontiguousarray(w_out[:, cols].T).astype(MDT_NP),
            "cos_t": cos128,
            "sin_t": sin128,
            "tri": tri.astype(MDT_NP),
            "oner": np.ones(1, dtype=MDT_NP),
            "epsc": np.full(1, RMS_EPS, dtype=np.float32),
        })
    return in_maps


def assemble(x, results):
    x = np.asarray(x, dtype=np.float32)
    b0 = sum(results[i]["yT"].astype(np.float32) for i in range(4))
    b1 = sum(results[i]["yT"].astype(np.float32) for i in range(4, 8))
    out = np.empty_like(x)
    out[0] = x[0] + b0.T
    out[1] = x[1] + b1.T
    return out


_PROGRAMS = {}


def _get_program(S):
    if S not in _PROGRAMS:
        _PROGRAMS[S] = build_program(S)
    return _PROGRAMS[S]


def run(x, w_in, w_out, rms_w, trace=False):
    from concourse.bass_utils import run_bass_kernel_spmd
    nc = _get_program(x.shape[1])
    in_maps = host_prepare(x, w_in, w_out, rms_w)
    res = run_bass_kernel_spmd(nc, in_maps, list(range(NCORES)), trace=trace)
    return assemble(x, res.results), res


def kernel(x, w_in, w_out, rms_w):
    out, _ = run(np.asarray(x), np.asarray(w_in), np.asarray(w_out),
                 np.asarray(rms_w))
    return out


# revision 7
# speedup vs baseline: 2.0942x; 2.0942x over previous
"""GroupedQueryAttention Trainium2 kernel (8-core SPMD), v2.

Reference op: RMSNorm -> in-proj (q/k/v) -> RoPE -> causal GQA attention
-> out-proj -> residual.  b=2, s=2048, d_model=2048, 32 q-heads / 8 KV
groups, head dim 64, fp32.

Sharding: core c handles batch b = c//4 and KV groups (2j, 2j+1), j = c%4
(data parallel over batch x tensor parallel over KV groups, Megatron
style).  Each core computes the full in-projection restricted to its 8
heads' channels, attention for its 8 heads, and a partial out-projection
(row-parallel).  The host sums the 4 partials per batch (f16 partials,
f32 sum) and adds the residual.

v2 structure (vs the v1 baseline):
  * Three-stage software pipeline: chunk c runs in-proj(c), attention
    (c-1) and out-proj(c-2) concurrently, with the PE instruction stream
    explicitly woven so matmuls from in/out-proj fill the gaps where
    attention waits on the ACT engine's exp.
  * PSUM plan (8 banks): ip 1 + qk 4 (double-buffered [128,2,512]) +
    av 2 + op 1.  The RMS sum-of-squares psum rides the qk tag early;
    the out-proj tail ping-pongs op/qk tags once attention is done.
  * Engine balance: exp on ACT; causal tri-masking on GPSIMD (otherwise
    idle); everything f16 on DVE where possible (2x rate); softmax
    reciprocals via reciprocal_approx_fast straight off the PSUM
    denominator rows; RMS norm uses one Sqrt phase up front so the ACT
    table set switches exactly once (sqrt -> exp).
  * RoPE as 3 DVE ops per tile: the rotate-half operand is produced by
    4 small SBUF->SBUF partition-shift DMAs instead of 4 extra DVE ops,
    with the sign pattern folded into the sin table.
  * w_out resident in SBUF; output written f16 (halves the store
    traffic; host sums partials in f32).
"""

import numpy as np
from collections import deque
from contextlib import ExitStack

import concourse.bass as bass
from concourse import bacc as _bacc
import concourse.mybir as mybir
import concourse.tile as tile
from concourse.bass import ts

import os
f32 = mybir.dt.float32
f16 = mybir.dt.float16
MDT = {"f16": f16, "bf16": mybir.dt.bfloat16}[os.environ.get("GQA_MM_DT", "f16")]
try:
    import ml_dtypes
    _BF16_NP = ml_dtypes.bfloat16
except ImportError:
    _BF16_NP = None
MDT_NP = {f16: np.float16, mybir.dt.bfloat16: _BF16_NP}[MDT]
AF = mybir.ActivationFunctionType
ALU = mybir.AluOpType

TRI_ENGINE = os.environ.get("GQA_TRI_ENGINE", "gpsimd")

D = 2048          # model dim
CH = 768          # per-core in-proj channels (8 q heads + 2 k + 2 v)
TOKC = 512        # token chunk
NKT = D // 128    # 16 k-tiles over model dim
RMS_EPS = 1e-6
ROPE_THETA = 10000.0
NCORES = 8


def build_program(S=2048):
    NCH = S // TOKC          # token chunks
    NSK = S // 128           # sk tiles
    nc = _bacc.Bacc(None)

    xT_d = nc.dram_tensor("xT", [D, S], MDT, kind="ExternalInput")
    w_inT_d = nc.dram_tensor("w_inT", [D, CH], MDT, kind="ExternalInput")
    w_outT_d = nc.dram_tensor("w_outT", [512, D], MDT, kind="ExternalInput")
    cos_d = nc.dram_tensor("cos_t", [128, S], MDT, kind="ExternalInput")
    sin_d = nc.dram_tensor("sin_t", [128, S], MDT, kind="ExternalInput")
    tri_d = nc.dram_tensor("tri", [128, 128], MDT, kind="ExternalInput")
    oner_d = nc.dram_tensor("oner", [1], MDT, kind="ExternalInput")
    eps_d = nc.dram_tensor("epsc", [1], f32, kind="ExternalInput")
    yT_d = nc.dram_tensor("yT", [D, S], MDT, kind="ExternalOutput")

    mask_eng = None  # set inside

    with tile.TileContext(nc) as tc, ExitStack() as ctx:
        sb = ctx.enter_context(tc.tile_pool(name="sb", bufs=1))
        sbs = ctx.enter_context(tc.tile_pool(name="sbs", bufs=2))
        dramp = ctx.enter_context(tc.tile_pool(name="dram", bufs=1, space="DRAM"))

        mask_eng = nc.gpsimd if TRI_ENGINE == "gpsimd" else nc.vector

        # ------------- persistent SBUF -------------
        w_in_sb = sb.tile([128, NKT, CH], MDT, name="w_in_sb")
        w_out_sb = sb.tile([128, 4, D], MDT, name="w_out_sb")
        x_sb = sb.tile([128, NCH * NKT, TOKC], MDT, name="x_sb")
        qkv = sb.tile([128, 6, S], MDT, name="qkv")     # ch tiles 0-3 q pairs, 4 k, 5 v
        oT = sb.tile([128, 4, S], MDT, name="oT")
        vA = sb.tile([128, NSK, 65], MDT, name="vA")    # V + ones col, group 0
        vB = sb.tile([128, NSK, 65], MDT, name="vB")    # group 1
        tri_sb = sb.tile([128, 128], MDT, name="tri_sb")
        ones_sb = sb.tile([128, 1], MDT, name="ones_sb")
        eps_sb = sb.tile([1, 1], f32, name="eps_sb")
        invT = sb.tile([128, NCH, TOKC // 128], f32, name="invT")
        inv128h = sb.tile([128, NCH, TOKC], MDT, name="inv128h")

        nrm_dr = dramp.tile([NCH, TOKC], f32, name="nrm_dr")
        nrm2h_dr = dramp.tile([NCH, TOKC], MDT, name="nrm2h_dr")
        db2_dr = dramp.tile([NCH, 4, 2, TOKC], f32, name="db2_dr")

        # ------------- static loads -------------
        # Two HWDGE queues (sync + scalar) run the bulk loads in parallel;
        # the scalar queue is otherwise idle until the first exp wave.
        w_inT_v = w_inT_d.rearrange("(o p) c -> p o c", p=128)
        for kt in range(NKT):
            nc.sync.dma_start(w_in_sb[:, kt, :], w_inT_v[:, kt, :])
        for kt in range(NKT):
            nc.scalar.dma_start(x_sb[:, kt, :], xT_d[ts(kt, 128), 0:TOKC])
        nc.sync.dma_start(tri_sb[:], tri_d[:])
        nc.sync.dma_start(ones_sb[:], oner_d[None, :].to_broadcast((128, 1)))
        nc.sync.dma_start(vA[:, :, 64:65], oner_d[None, None, :].to_broadcast((128, NSK, 1)))
        nc.sync.dma_start(vB[:, :, 64:65], oner_d[None, None, :].to_broadcast((128, NSK, 1)))
        nc.sync.dma_start(eps_sb[:], eps_d[None, :])
        w_outT_v = w_outT_d.rearrange("(o p) c -> p o c", p=128)
        for kt in range(4):
            nc.sync.dma_start(w_out_sb[:, kt, :], w_outT_v[:, kt, :])
        for c in range(1, NCH):
            for kt in range(NKT):
                eng = nc.scalar if (kt % 2 == 0) else nc.sync
                eng.dma_start(x_sb[:, c * NKT + kt, :],
                              xT_d[ts(kt, 128), c * TOKC:(c + 1) * TOKC])

        with tc.tile_pool(name="ps", bufs=1, space="PSUM") as ps:

            # ---------------- RMS-norm chain for one chunk ----------------
            def ss_items(c):
                """PE filler items (2 matmuls each) for chunk c's sum of
                squares, plus the trailing scalar chain."""
                ss = ps.tile([1, TOKC], f32, tag="qk", bufs=2, name=f"ss_{c}")

                def mk(j):
                    def emit():
                        for kt in (2 * j, 2 * j + 1):
                            xsq = sbs.tile([128, TOKC], MDT, tag="xsq", bufs=2,
                                           name=f"xsq_{c}_{kt}")
                            nc.vector.tensor_tensor(
                                xsq[:], x_sb[:, c * NKT + kt, :],
                                x_sb[:, c * NKT + kt, :], ALU.mult)
                            nc.tensor.matmul(ss[:], ones_sb[:], xsq[:],
                                             start=(kt == 0), stop=(kt == NKT - 1))
                    return emit

                def post():
                    # inv_rms = 1/sqrt(ss/D + eps); reciprocal runs in a
                    # [128, 4] token-partition layout via a DRAM bounce.
                    sqm = sbs.tile([1, TOKC], f32, tag="sqm", bufs=2, name=f"sqm_{c}")
                    nc.scalar.activation(sqm[:], ss[:], AF.Sqrt,
                                         bias=eps_sb[:], scale=1.0 / D)
                    nc.sync.dma_start(nrm_dr[c][None, :], sqm[:])
                    srT = sbs.tile([128, TOKC // 128], f32, tag="srT", bufs=2,
                                   name=f"srT_{c}")
                    nc.sync.dma_start(srT[:], nrm_dr[c].rearrange("(a p) -> p a", p=128))
                    nc.vector.reciprocal_approx_fast(invT[:, c, :], srT[:])
                    invTh = sbs.tile([128, TOKC // 128], MDT, tag="invTh", bufs=2,
                                     name=f"invTh_{c}")
                    nc.vector.tensor_copy(invTh[:], invT[:, c, :])
                    nc.sync.dma_start(nrm2h_dr[c].rearrange("(a p) -> p a", p=128),
                                      invTh[:])
                    nc.sync.dma_start(inv128h[:, c, :],
                                      nrm2h_dr[c][None, :].to_broadcast((128, TOKC)))

                return [mk(j) for j in range(NKT // 2)], post

            # ---------------- in-projection m-tile ----------------
            def ip_items(c, m):
                cs = slice(c * TOKC, (c + 1) * TOKC)
                ip = ps.tile([128, TOKC], f32, tag="ip", bufs=1, name=f"ip{m}_{c}")

                def mk(j):
                    def emit():
                        for kt in (2 * j, 2 * j + 1):
                            nc.tensor.matmul(ip[:], w_in_sb[:, kt, ts(m, 128)],
                                             x_sb[:, c * NKT + kt, :],
                                             start=(kt == 0), stop=(kt == NKT - 1))
                    return emit

                def post():
                    nc.vector.tensor_copy(qkv[:, m, cs], ip[:])
                    if m < 5:
                        # rope in place: rotate-half operand via partition-shift
                        # DMAs, sign pattern folded into the sin table.
                        qs = sbs.tile([128, TOKC], MDT, tag="qs", bufs=2,
                                      name=f"qs_{c}_{m}")
                        for dst, src in ((0, 32), (32, 0), (64, 96), (96, 64)):
                            nc.sync.dma_start(qs[dst:dst + 32, :],
                                              qkv[src:src + 32, m, cs])
                        rtmp = sbs.tile([128, TOKC], MDT, tag="rtmp", bufs=2,
                                        name=f"rtmp_{c}_{m}")
                        nc.vector.tensor_tensor(rtmp[:], qs[:], sini_t[c][:], ALU.mult)
                        nc.vector.tensor_tensor(qkv[:, m, cs], qkv[:, m, cs],
                                                cosi_t[c][:], ALU.mult)
                        nc.vector.tensor_tensor(qkv[:, m, cs], qkv[:, m, cs],
                                                rtmp[:], ALU.add)
                    else:
                        # V: transpose to [token, dv] (+ 1/rms per-token scale)
                        for tl in range(TOKC // 128):
                            t = c * (TOKC // 128) + tl
                            vtt = sbs.tile([128, 128], MDT, tag="vtt", bufs=2,
                                           name=f"vtt_{t}")
                            nc.sync.dma_start(vtt[:], qkv[:, 5, ts(t, 128)],
                                              transpose=True)
                            nc.scalar.activation(vA[:, t, 0:64], vtt[:, 0:64],
                                                 AF.Copy, scale=invT[:, c, tl:tl + 1])
                            nc.scalar.activation(vB[:, t, 0:64], vtt[:, 64:128],
                                                 AF.Copy, scale=invT[:, c, tl:tl + 1])

                return [mk(j) for j in range(NKT // 2)], post

            # ---------------- out-projection m-tile ----------------
            def op_item(o, m, tag="op"):
                os_ = slice(o * TOKC, (o + 1) * TOKC)

                def emit():
                    op = ps.tile([128, TOKC], f32, tag=tag,
                                 bufs=(1 if tag == "op" else 2), name=f"op_{o}_{m}")
                    for kt in range(4):
                        nc.tensor.matmul(op[:], w_out_sb[:, kt, ts(m, 128)],
                                         oT[:, kt, os_],
                                         start=(kt == 0), stop=(kt == 3))
                    yt = sbs.tile([128, TOKC], MDT, tag="yt", bufs=3,
                                  name=f"yt_{o}_{m}")
                    nc.vector.tensor_copy(yt[:], op[:])
                    nc.sync.dma_start(yT_d[ts(m, 128), os_], yt[:])
                return emit

            # ---------------- attention pair (spine) ----------------
            def attn_pair(a, p, items):
                """Emit the attention chain for pair p of chunk a, draining
                `items` (a deque of PE-work closures) between the QK and AV
                matmul groups so the PE never idles on the ACT exp."""
                cs = slice(a * TOKC, (a + 1) * TOKC)
                n_t = 4 * (a + 1)
                avA = ps.tile([65, TOKC], f32, tag="av", bufs=2, name=f"avA_{a}_{p}")
                avB = ps.tile([65, TOKC], f32, tag="av", bufs=2, name=f"avB_{a}_{p}")
                for t in range(n_t):
                    j0 = max(0, t - 4 * a) * 128
                    qk = ps.tile([128, 2, TOKC], f32, tag="qk", bufs=2,
                                 name=f"qk_{a}_{p}_{t}")
                    # the pair's two heads: row-tiled concurrent K=64 matmuls
                    nc.tensor.matmul(
                        qk[:, 0, j0:],
                        qkv[0:64, 4, ts(t, 128)],
                        qkv[0:64, p, a * TOKC + j0:(a + 1) * TOKC],
                        start=True, stop=True,
                    )
                    nc.tensor.matmul(
                        qk[:, 1, j0:],
                        qkv[64:128, 4, ts(t, 128)],
                        qkv[64:128, p, a * TOKC + j0:(a + 1) * TOKC],
                        start=True, stop=True,
                    )
                    e = sbs.tile([128, 2, TOKC], MDT, tag="e", bufs=3,
                                 name=f"e_{a}_{p}_{t}")
                    nc.scalar.activation(e[:, :, j0:], qk[:, :, j0:], AF.Exp)
                    if t >= 4 * a:  # diagonal tile: causal mask
                        for h in (0, 1):
                            mask_eng.tensor_tensor(
                                e[:, h, j0:j0 + 128],
                                e[:, h, j0:j0 + 128],
                                tri_sb[:],
                                ALU.mult,
                            )
                    # drain filler so the PE has work while ACT runs exp
                    k = -(-len(items) // (n_t - t)) if items else 0
                    for _ in range(min(k, 2 + (len(items) > 2 * (n_t - t)))):
                        if items:
                            items.popleft()()
                    nc.tensor.matmul(avA[:, j0:], vA[:, t, :], e[:, 0, j0:],
                                     start=(t == 0), stop=(t == n_t - 1))
                    nc.tensor.matmul(avB[:, j0:], vB[:, t, :], e[:, 1, j0:],
                                     start=(t == 0), stop=(t == n_t - 1))
                # softmax denominators live in row 64 of each AV psum.
                d2i = sbs.tile([1, 2, TOKC], f32, tag="d2i", bufs=2,
                               name=f"d2i_{a}_{p}")
                if RECIP_SBUF:
                    dsb = sbs.tile([1, 2, TOKC], f32, tag="dsb", bufs=2,
                                   name=f"dsb_{a}_{p}")
                    nc.vector.tensor_copy(dsb[:, 0, :], avA[64:65, :])
                    nc.vector.tensor_copy(dsb[:, 1, :], avB[64:65, :])
                    nc.vector.reciprocal_approx_fast(d2i[:, 0, :], dsb[:, 0, :])
                    nc.vector.reciprocal_approx_fast(d2i[:, 1, :], dsb[:, 1, :])
                else:
                    nc.vector.reciprocal_approx_fast(d2i[:, 0, :], avA[64:65, :])
                    nc.vector.reciprocal_approx_fast(d2i[:, 1, :], avB[64:65, :])
                nc.sync.dma_start(db2_dr[a, p], d2i[0])
                dbAB = sbs.tile([128, TOKC], f32, tag="dbAB", bufs=2,
                                name=f"dbAB_{a}_{p}")
                nc.sync.dma_start(
                    dbAB[0:64, :], db2_dr[a, p, 0][None, :].to_broadcast((64, TOKC)))
                nc.sync.dma_start(
                    dbAB[64:128, :], db2_dr[a, p, 1][None, :].to_broadcast((64, TOKC)))
                # evacuate the AV psum immediately (frees the banks for the
                # next pair); the normalize is deferred until the broadcast
                # DMA has landed so the DVE queue never head-blocks on it.
                nc.vector.tensor_copy(oT[0:64, p, cs], avA[0:64, :])
                nc.vector.tensor_copy(oT[64:128, p, cs], avB[0:64, :])

                def norm():
                    nc.vector.tensor_tensor(oT[:, p, cs], oT[:, p, cs], dbAB[:],
                                            ALU.mult)
                return norm

            # ---------------- chunk-level schedule ----------------
            # cosi/sini per chunk (rope tables scaled by 1/rms)
            cosi_t = {}
            sini_t = {}

            def emit_chunk_tables(c):
                cos_c = sbs.tile([128, TOKC], MDT, tag="cos_c", bufs=2,
                                 name=f"cos_c_{c}")
                nc.sync.dma_start(cos_c[:], cos_d[:, c * TOKC:(c + 1) * TOKC])
                sin_c = sbs.tile([128, TOKC], MDT, tag="sin_c", bufs=2,
                                 name=f"sin_c_{c}")
                nc.sync.dma_start(sin_c[:], sin_d[:, c * TOKC:(c + 1) * TOKC])
                cosi = sbs.tile([128, TOKC], MDT, tag="cosi", bufs=2,
                                name=f"cosi_{c}")
                nc.vector.tensor_tensor(cosi[:], cos_c[:], inv128h[:, c, :], ALU.mult)
                sini = sbs.tile([128, TOKC], MDT, tag="sini", bufs=2,
                                name=f"sini_{c}")
                nc.vector.tensor_tensor(sini[:], sin_c[:], inv128h[:, c, :], ALU.mult)
                cosi_t[c] = cosi
                sini_t[c] = sini

            # chunk-0 norm chain runs first (gates everything)
            items0, post0 = ss_items(0)
            for it in items0:
                it()
            post0()
            emit_chunk_tables(0)

            # op m-tiles per step: 16 spread over 6 steps
            OP_STEPS = [range(0, 3), range(3, 6), range(6, 9), range(9, 12),
                        range(12, 14), range(14, 16)]

            pend_ip = None      # in-proj psum evac + rope of the previous step
            pend_norm = None    # deferred oT normalize of the previous pair
            for c in range(NCH + 2):
                a = c - 1   # attention chunk
                o = c - 2   # out-proj chunk
                if 0 < c < NCH:
                    emit_chunk_tables(c)
                for s in range(6):
                    # previous step's deferred DVE work goes first in the
                    # queue: the evac frees the single ip psum bank before
                    # this step's first in-proj matmul needs it.
                    if pend_ip is not None:
                        pend_ip()
                        pend_ip = None
                    if pend_norm is not None:
                        pend_norm()
                        pend_norm = None
                    ip_list = []
                    op_list = []
                    posts = []
                    if c < NCH:
                        ip_list, post = ip_items(c, s)
                        pend_ip = post
                    if c == 0 and 1 <= s <= 3:
                        its, post = ss_items(s)
                        ip_list.extend(its)
                        posts.append(post)
                    if 0 <= o < NCH and c < NCH + 1:
                        op_list = [op_item(o, m) for m in OP_STEPS[s]]
                    if c == NCH + 1:
                        # tail: out-proj of the last chunk ping-pongs between
                        # the op and (now free) qk psum tags
                        op_list = [op_item(o, m, tag=("op" if (m % 2 == 0)
                                                      else "qk"))
                                   for m in OP_STEPS[s]]
                    # merge, spreading op psum chains out between ip groups
                    items = deque()
                    if not op_list:
                        items.extend(ip_list)
                    else:
                        k, stride = 0, max(1, len(ip_list) // len(op_list))
                        for ob in op_list:
                            items.extend(ip_list[k:k + stride])
                            k += stride
                            items.append(ob)
                        items.extend(ip_list[k:])
                    if 0 <= a < NCH and s < 4:
                        pend_norm = attn_pair(a, s, items)
                    while items:
                        items.popleft()()
                    for post in posts:
                        post()
                    if pend_ip is not None and c < NCH and s == 5:
                        # V-copies must be emitted before the next chunk's
                        # attention starts consuming vA/vB
                        pend_ip()
                        pend_ip = None

    nc.finalize()
    return nc


# ------------------------------- host side ----------------------------------

def _rope_tables(S):
    inv_freq = ROPE_THETA ** (-np.arange(0, 64, 2, dtype=np.float64) / 64.0)  # [32]
    ang = np.arange(S, dtype=np.float64)[:, None] * inv_freq[None, :]          # [S, 32]
    cosb = np.cos(ang).T.astype(np.float32)   # [32, S]
    sinb = np.sin(ang).T.astype(np.float32)
    cos128 = np.tile(cosb, (4, 1))                                             # [128, S]
    # sign pattern for the pre-swapped rotate-half operand:
    # out[0:32] needs -x2*s, out[32:64] needs +x1*s (per 64-row head block)
    sin128 = np.concatenate([-sinb, sinb, -sinb, sinb], axis=0)                # [128, S]
    return (np.ascontiguousarray(cos128).astype(MDT_NP),
            np.ascontiguousarray(sin128).astype(MDT_NP))


def host_prepare(x, w_in, w_out, rms_w):
    """Build the 8 per-core input maps."""
    S = x.shape[1]
    x = np.asarray(x, dtype=np.float32)
    w_eff = np.asarray(w_in, dtype=np.float32) * np.asarray(rms_w, np.float32)[None, :]
    w_out = np.asarray(w_out, dtype=np.float32)
    cos128, sin128 = _rope_tables(S)
    tri = np.ascontiguousarray(np.triu(np.ones((128, 128), dtype=np.float32)))
    qscale = np.float32(64 ** -0.5)

    in_maps = []
    for core in range(NCORES):
        b, j = divmod(core, 4)
        g0, g1 = 2 * j, 2 * j + 1
        rows = []
        for p in range(4):
            for g in (g0, g1):
                rows.extend(range((g * 4 + p) * 64, (g * 4 + p) * 64 + 64))
        for g in (g0, g1):
            rows.extend(range(2048 + g * 64, 2048 + g * 64 + 64))
        for g in (g0, g1):
            rows.extend(range(2560 + g * 64, 2560 + g * 64 + 64))
        w_slice = w_eff[rows, :].copy()
        w_slice[:512, :] *= qscale
        cols = []
        for p in range(4):
            for g in (g0, g1):
                cols.extend(range((g * 4 + p) * 64, (g * 4 + p) * 64 + 64))
        in_maps.append({
            "xT": np.ascontiguousarray(x[b].T).astype(MDT_NP),
            "w_inT": np.ascontiguousarray(w_slice.T).astype(MDT_NP),
            "w_outT": np.ascontiguousarray(w_out[:, cols].T).astype(MDT_NP),
            "cos_t": cos128,
            "sin_t": sin128,
            "tri": tri.astype(MDT_NP),
            "oner": np.ones(1, dtype=MDT_NP),
            "epsc": np.full(1, RMS_EPS, dtype=np.float32),
        })
    return in_maps


def assemble(x, results):
    x = np.asarray(x, dtype=np.float32)
    b0 = sum(results[i]["yT"].astype(np.float32) for i in range(4))
    b1 = sum(results[i]["yT"].astype(np.float32) for i in range(4, 8))
    out = np.empty_like(x)
    out[0] = x[0] + b0.T
    out[1] = x[1] + b1.T
    return out


_PROGRAMS = {}


def _get_program(S):
    if S not in _PROGRAMS:
        _PROGRAMS[S] = build_program(S)
    return _PROGRAMS[S]


def run(x, w_in, w_out, rms_w, trace=False):
    from concourse.bass_utils import run_bass_kernel_spmd
    nc = _get_program(x.shape[1])
    in_maps = host_prepare(x, w_in, w_out, rms_w)
    res = run_bass_kernel_spmd(nc, in_maps, list(range(NCORES)), trace=trace)
    return assemble(x, res.results), res


def kernel(x, w_in, w_out, rms_w):
    out, _ = run(np.asarray(x), np.asarray(w_in), np.asarray(w_out),
                 np.asarray(rms_w))
    return out
